# revision 41
# baseline (speedup 1.0000x reference)
"""Block-causal GQA attention on 8 trn2 NeuronCores.

Sharding: core = b*4 + g  (b in {0,1} batch, g in {0..3} kv-head group).
Each core computes, for its batch b and kv group g (4 q-heads, 1 kv head):
    partial_out = softmax_blockcausal(rope(x@Wq_g) @ rope(x@Wk_g)^T) @ (x@Wv_g) @ Wo_g
Host sums the 4 group partials per batch (partials written bf16, summed f32).

Device design (bf16 matmuls, f32 PSUM), single fused schedule keeping PE and
ACT concurrently busy:
  - Host passes x^T, so Q^T/K^T/V^T come out of projections with d on
    partitions; RoPE (sign folded into the sin table) on DVE during PSUM
    eviction.  V^T is DMA-xbar-transposed into V_aug = [V | ones].
  - The K + FULL V projection (8 matmuls/chunk) rides the x^T wave
    c-outer across all 8 psum banks (V j2/3 in the s0 pair); x^T splits
    over the sync (even chunks) and gpsimd (odd) queues, wk/wv stream in
    4-chunk pieces on scalar, and chunks 0/1 land in halves, so the first
    real matmul starts ~11.5us (DMA floor) with pre-wave warms covering
    PE boot.  The wave is HBM-paced (~310GB/s) until ~18us, PE-paced after.
  - Deferred weight DMAs, all gated by probe writes on x^T chunk LANDING
    tiles (queues are strict FIFO — gating on an eviction fired ~25us late
    once the scheduler buried that DVE chain): cos/sin/wq0 on the scalar
    queue after chunk 10 (they must beat the K evictions at the wave's PE
    end ~44us); wq1-3 + Wo on gpsimd after chunk 15.
  - Phase B: attention blocks (2,0),(3,0),(0,1),(1,1),(2,1),(3,1), each
    S^T+exp window packed tk-by-tk with PE filler matched to the exp
    widths: Y([Y|Z] += P^T_tile.T @ V_aug) groups largest-first, O-proj
    rows spread as quarter-row psum chunks (block (0,1) filler starts at
    tk=2 with the y(3,0) stragglers at tk=0/1 — its first cycles are
    otherwise PE-underfed).  S^T psums are 512-wide from a 5-deep ring
    (exp of a tile's first half overlaps its second half's matmul); the
    [Y|Z]/transpose/O-chunk psums share a 3-deep b1 ring.
  - O[t,n] = sum_h Y_h^T.T @ Wo_h accumulated in PSUM over heads; partials
    evicted bf16.  In-block rows DMA per-chunk from gpsimd; tail rows from
    sync; rows 14/15 collect into row buffers with half-row DMAs issued
    right after the covering evict, and the kernel's last PE work is a
    128-col sliver (ACT-evicted, scalar-queue DMA) so the end-of-context
    barrier — which waits on every queue's drain, gpsimd's being ~5us if
    it still holds late work — closes ~1us after the last matmul retires.
  - Host sums the 4 group partials per batch in f32.
  - Measured (core 0, full clock): ~236us HW exec (down from ~250us), PE
    busy ~215us vs a ~198us pure-stream floor.  The part clock varies run
    to run: 512-wide MM median 216ns = warm 2.4GHz, 259ns = P0 downclock
    (rerun it), 427ns = HAM-cold — compare only same-clock runs.
"""
import os
import sys
import numpy as np

for _p in ("/opt/trn_rl_repo",):
    if _p not in sys.path and os.path.isdir(_p):
        sys.path.insert(0, _p)

import ml_dtypes

BF16 = ml_dtypes.bfloat16

B = 2
T = 2048
C = 2048
HD = 128
NHL = 4           # q heads per core
NT = T // 128     # 16 query/key tiles
NCH = C // 128    # 16 contraction chunks
HW = T // 2       # tq half width
SCALE = 1.0 / float(np.sqrt(np.float32(HD)))

_CACHE = {}


def _build_nc():
    import concourse.bass as bass
    import concourse.mybir as mybir
    import concourse.tile as tile
    from concourse import bacc
    from concourse import masks

    dt = mybir.dt
    f32 = dt.float32
    bf = dt.bfloat16
    Exp = mybir.ActivationFunctionType.Exp

    nc = bacc.Bacc(None, target_bir_lowering=False)

    # weights host-prelaid as [partition, chunk, m] so each DMA is 128 fat
    # contiguous descriptors instead of 2048 small ones
    xT = nc.declare_dram_parameter("xT", [C, T], bf, isOutput=False)
    wq = nc.declare_dram_parameter("wq", [128, NHL, NCH, HD], bf, isOutput=False)
    wk = nc.declare_dram_parameter("wk", [128, NCH, HD], bf, isOutput=False)
    wv = nc.declare_dram_parameter("wv", [128, NCH, HD], bf, isOutput=False)
    wo = nc.declare_dram_parameter("wo", [128, NHL, C], bf, isOutput=False)
    cosT = nc.declare_dram_parameter("cosT", [HD, T], bf, isOutput=False)
    sinT = nc.declare_dram_parameter("sinT", [HD, T], bf, isOutput=False)
    o = nc.declare_dram_parameter("o_part", [T, C], bf, isOutput=True)

    with tile.TileContext(nc) as tc:
        with tc.tile_pool(name="consts", bufs=1) as consts:
            # ---- persistent tiles (survive into phase B) ----
            # V_aug = [V | ones]: col 128 preset to 1, cols 0:128 filled by
            # DMA-transpose from V^T after the V projection.  Rows are 256
            # wide so each tile's dst offset stays 512B-aligned — the DMA
            # xbar transpose corrupts data at unaligned dst offsets.
            vaug_sb = consts.tile([128, NT, 2 * HD], bf, name="vaug_sb")
            nc.vector.memset(vaug_sb[:, :, HD:HD + 1], 1.0)

            ident = consts.tile([128, 128], bf, name="ident")
            masks.make_identity(nc, ident)

            # warm the ACT exp table set before phase A needs it
            dumm = consts.tile([1, 8], f32, name="dumm")
            nc.vector.memset(dumm, 0.0)
            nc.scalar.activation(dumm, dumm, Exp)

            wo_sb = consts.tile([128, NHL, C], bf, name="wo_sb")
            kt_sb = consts.tile([128, T], bf, name="kt_sb")
            qt_sb = [consts.tile([128, T], bf, name=f"qt{h}") for h in range(NHL)]
            yt_sb = [consts.tile([128, T], bf, name=f"yt{h}") for h in range(NHL)]

            # exp'd S^T tiles produced during phase A (heads 0,1 half 0);
            # consumed by phase B's first Y rounds
            p0_sb = [[consts.tile([128, HW], bf, name=f"p0_{h}_{tk}")
                      for tk in range(8)] for h in range(2)]

            # ============ phase A: projections + early attention ==========
            with tc.tile_pool(name="xtp", bufs=1) as xtp, \
                 tc.tile_pool(name="proj", bufs=1) as proj, \
                 tc.tile_pool(name="psA", bufs=1, space="PSUM") as pA:

                # phase-A-only SBUF (freed before phase B)
                wk_sb = proj.tile([128, NCH, HD], bf, name="wk_sb")
                wv_sb = proj.tile([128, NCH, HD], bf, name="wv_sb")
                wq_sb = proj.tile([128, NHL, NCH, HD], bf, name="wq_sb")
                cos_sb = proj.tile([128, T], bf, name="cos_sb")
                sin_sb = proj.tile([128, T], bf, name="sin_sb")
                vt_sb = proj.tile([128, T], bf, name="vt_sb")

                # HBM is the constraint during the wave: only wk/wv (needed
                # by the first matmul) go up front (scalar queue, parallel
                # with the x^T stream on sync).  Everything else issues from
                # gpsimd AFTER chunk 12 lands (dummy-read dependency), so
                # the wave owns the full bandwidth.
                # wk/wv stream in interleaved 4-chunk pieces so chunk 0's
                # first matmul waits on 128KB, not the whole megabyte
                for i in range(4):
                    nc.scalar.dma_start(wk_sb[:, 4 * i:4 * (i + 1)],
                                        wk[:, 4 * i:4 * (i + 1), :])
                    nc.scalar.dma_start(wv_sb[:, 4 * i:4 * (i + 1)],
                                        wv[:, 4 * i:4 * (i + 1), :])

                xt_r = xT.rearrange("(n p) t -> n p t", p=128)
                xt_sb = []
                for cch in range(NCH):
                    xt_c = xtp.tile([128, T], bf, name=f"xt{cch}")
                    # two queues; the first chunk on each queue lands in
                    # halves so its first matmuls start ~2us sooner.
                    # (chunk 15 prefetched on scalar was tried: it steals
                    # early-wave HBM and costs more at the head than it
                    # saves at the wave end)
                    q = nc.sync if cch % 2 == 0 else nc.gpsimd
                    if cch < 2:
                        q.dma_start(xt_c[:, 0:1024], xt_r[cch][:, 0:1024])
                        q.dma_start(xt_c[:, 1024:T], xt_r[cch][:, 1024:T])
                    else:
                        q.dma_start(xt_c, xt_r[cch])
                    xt_sb.append(xt_c)

                # gate the remaining input DMAs behind wave progress.  The
                # scheduler orders by readiness, not emission, so a plain
                # copy "before" the dma_start would be hoisted past it —
                # instead write a probe INTO each DMA's destination tile so
                # the DMA has a real WAW dependency on the gate.
                def gated_dma(dst_probe, dst, src, probe, q=None):
                    nc.gpsimd.tensor_copy(dst_probe, probe)
                    (q or nc.gpsimd).dma_start(dst, src)

                # (gating wv behind chunk 0 was tried: the early V matmuls
                # then wait ~2.5us on the full wv tile — net +5us)
                # cos/sin/wq0 ride the SCALAR queue (idle after wk/wv and
                # NOT behind the x chunks — a queue is FIFO, so putting
                # them on sync would wait for chunk 14 regardless of the
                # gate), gated on chunk 10: the wave's PE end (~43us)
                # trails its DMA stream by ~5us, so injecting 1.5MB here
                # delays chunks 11-15 harmlessly while cos/sin land
                # BEFORE the K evictions need them
                probe1 = xt_sb[10][0:1, 0:8]
                gated_dma(cos_sb[0:1, 0:8], cos_sb, cosT[:, :], probe1,
                          q=nc.scalar)
                gated_dma(sin_sb[0:1, 0:8], sin_sb, sinT[:, :], probe1,
                          q=nc.scalar)
                gated_dma(wq_sb[0:1, 0, 0, 0:8], wq_sb[:, 0], wq[:, 0],
                          probe1, q=nc.scalar)

                def rope_evict(ps, jsl, dst, dve=False):
                    # dst[:, jsl] = ps * cos + rot_half(ps) * sin  (bf16).
                    # ACT does the PSUM eviction (or DVE, for the K wave's
                    # back-to-back evictions, so the four chains don't
                    # serialize on one queue); DVE runs at bf16 2x.
                    t0 = proj.tile([128, 512], bf, tag="t0", bufs=4)
                    t1 = proj.tile([128, 512], bf, tag="t1", bufs=4)
                    t2 = proj.tile([128, 512], bf, tag="t2", bufs=4)
                    # sin table halves are pre-swapped on host so each mul
                    # reads both SBUF inputs at the same base partition
                    # (walrus requires equal SBUF base partitions).
                    if dve:
                        nc.vector.tensor_copy(t0, ps)
                    else:
                        nc.scalar.copy(t0, ps)
                    nc.vector.tensor_mul(t1, t0, cos_sb[:, jsl])
                    nc.vector.tensor_mul(t2[0:64], t0[64:128], sin_sb[64:128, jsl])
                    nc.vector.tensor_mul(t2[64:128], t0[0:64], sin_sb[0:64, jsl])
                    nc.vector.tensor_add(dst[:, jsl], t1, t2)

                # V j=2,3 accumulate c-outer in the s0 psum banks (idle
                # until the first woven S^T tile, which comes long after
                # the wave evictions).  Warm matmuls target its first rows
                # pre-wave (WAW-ordered before the start=True clear).
                ps_v23 = pA.tile([128, HW], f32, tag="s0", bufs=1,
                                 name="ps_v23")

                def warm_mm(n):
                    # both operands read never-written SBUF (garbage) so
                    # the first warm has zero dependencies and starts as
                    # soon as the PE sequencer is up
                    for _ in range(n):
                        nc.tensor.matmul(ps_v23[0:1, 0:512],
                                         kt_sb[:, 0:1],
                                         kt_sb[:, 0:512], start=True, stop=True)

                # all warms run PRE-wave (they WAW-order before chunk 0's
                # start=True clear of the same psum rows): just enough to
                # cover PE boot (~6.6us) through chunk 0 half-0's landing
                # (~11.5us) so the first real matmuls run at full clock
                warm_mm(8)

                # -- wave 1: K and V over full T, c-outer, 8 psum banks so
                #    the whole 1.73us/chunk of projection work rides the
                #    DMA-paced wave (no throwaway filler needed mid-wave) --
                ps_k = [pA.tile([128, 512], f32, tag="pj", bufs=6,
                                name=f"ps_k{j}") for j in range(4)]
                ps_v = [pA.tile([128, 512], f32, tag="pj", bufs=6,
                                name=f"ps_v{j}") for j in range(2)]
                for cch in range(NCH):
                    st, sp = (cch == 0), (cch == NCH - 1)
                    # half-major emission: the x^T-half-0 consumers (K/V
                    # j0,j1) go first so a chunk's matmuls can start as
                    # soon as its first half lands
                    for j in (0, 1, 2, 3):
                        jsl = slice(512 * j, 512 * (j + 1))
                        nc.tensor.matmul(ps_k[j], wk_sb[:, cch, :],
                                         xt_sb[cch][:, jsl], start=st, stop=sp)
                        if j < 2:
                            nc.tensor.matmul(ps_v[j], wv_sb[:, cch, :],
                                             xt_sb[cch][:, jsl],
                                             start=st, stop=sp)
                        else:
                            psl = slice(512 * (j - 2), 512 * (j - 1))
                            nc.tensor.matmul(ps_v23[:, psl], wv_sb[:, cch, :],
                                             xt_sb[cch][:, jsl],
                                             start=st, stop=sp)
                for j in range(4):
                    jsl = slice(512 * j, 512 * (j + 1))
                    rope_evict(ps_k[j], jsl, kt_sb, dve=(j % 2 == 1))
                for j in range(2):
                    jsl = slice(512 * j, 512 * (j + 1))
                    nc.vector.tensor_copy(vt_sb[:, jsl], ps_v[j])
                for j in range(2, 4):
                    jsl = slice(512 * j, 512 * (j + 1))
                    psl = slice(512 * (j - 2), 512 * (j - 1))
                    nc.vector.tensor_copy(vt_sb[:, jsl], ps_v23[:, psl])
                # second gate: the remaining weights start once the LAST
                # x^T chunk has landed — a pure-DMA signal.  (Gating on a
                # K eviction was tried: the scheduler sequenced that DVE
                # chain deep into the Q phase, so wq1's DMA started ~25us
                # late and q_block(1,0) stalled 5us cold.)
                probe2 = xt_sb[15][0:1, 0:8]
                for h in range(1, NHL):
                    gated_dma(wq_sb[0:1, h, 0, 0:8], wq_sb[:, h], wq[:, h],
                              probe2)
                gated_dma(wo_sb[0:1, 0, 0:8], wo_sb, wo[:, :, :], probe2)
                for i in range(NT):
                    nc.sync.dma_start_transpose(
                        vaug_sb[:, i, 0:HD], vt_sb[:, 128 * i:128 * (i + 1)]
                    )

                def q_block(h, j, pre=None, mid=None, mid2=None):
                    """One Q psum (c-inner) with optional callables emitted
                    before / midway through the chunk loop (weave points)."""
                    jsl = slice(512 * j, 512 * (j + 1))
                    ps_q = pA.tile([128, 512], f32, tag="pj", bufs=6,
                                   name=f"ps_q{h}_{j}")
                    if pre is not None:
                        pre()
                    for cch in range(NCH):
                        nc.tensor.matmul(
                            ps_q, wq_sb[:, h, cch, :], xt_sb[cch][:, jsl],
                            start=(cch == 0), stop=(cch == NCH - 1))
                        if cch == 7 and mid is not None:
                            mid()
                        if cch == 11 and mid2 is not None:
                            mid2()
                    rope_evict(ps_q, jsl, qt_sb[h])

                def s_tile_A(h, tk):
                    """Woven S^T + exp for (head h, half 0, key tile tk):
                    p0_sb[h][tk][:, lo:] = exp(K_tk^T.T @ Q^T * scale)."""
                    lo = 128 * tk
                    ps = pA.tile([128, HW], f32, tag="s0", bufs=1, name="ps_s0")
                    chunks = ([(lo, 512), (512, HW)] if lo < 512
                              else [(lo, HW)])
                    for (a, bnd) in chunks:
                        nc.tensor.matmul(
                            ps[:, a:bnd],
                            kt_sb[:, 128 * tk:128 * (tk + 1)],
                            qt_sb[h][:, a:bnd],
                            start=True, stop=True)
                    nc.scalar.activation(p0_sb[h][tk][:, lo:HW], ps[:, lo:HW],
                                         Exp, scale=SCALE)

                # -- Q head 0 (no attention weave yet) --
                q_block(0, 0)
                q_block(0, 1)
                q_block(0, 2)
                q_block(0, 3)

                # -- Q heads 1..3 with early S^T+exp woven in: head 0's
                #    half-0 tiles during Q(h1)/Q(h2), head 1's during Q(h3) --
                for j in range(4):
                    q_block(1, j, mid=lambda j=j: s_tile_A(0, j))
                for j in range(4):
                    q_block(2, j, mid=lambda j=j: s_tile_A(0, 4 + j))
                # head 1's eight tiles finish by Q(h3,j2) — the sooner the
                # last woven exp retires, the sooner phase B's first S^T
                # psum banks (which alias the s0 slot) are free
                h1s = [(0, 1, 2), (3, 4, 5), (6, None, 7), (None, None, None)]
                for j in range(4):
                    a, b, c = h1s[j]
                    q_block(3, j,
                            pre=(lambda a=a: s_tile_A(1, a))
                            if a is not None else None,
                            mid=(lambda b=b: s_tile_A(1, b))
                            if b is not None else None,
                            mid2=(lambda c=c: s_tile_A(1, c))
                            if c is not None else None)

                # bridge the A->B psum-pool transition with throwaway
                # matmuls from an already-free pj slot, so PE stays busy
                # (and HAM stays at full rate) while the last projection
                # psums drain and phase B's banks free up
                warm2 = pA.tile([128, 512], f32, tag="pj", bufs=6,
                                name="warm2")
                for _ in range(8):
                    nc.tensor.matmul(warm2[0:1, :],
                                     vaug_sb[:, 0, HD:HD + 1],
                                     kt_sb[:, 0:512], start=True, stop=True)

            # ============ phase B: attention + output projection ==========
            with tc.tile_pool(name="attn", bufs=1) as ap, \
                 tc.tile_pool(name="psB", bufs=1, space="PSUM") as pB:

                # p tiles for S^T produced in phase B (heads 2,3 half 0 and
                # all heads half 1)
                def s_tile_B(h, half, tk, ptile):
                    # 512-wide psum chunks from a 6-deep ring (vs one
                    # 1024-wide tile from a 3-deep ring): the exp of a
                    # tile's first half overlaps the matmul of its second,
                    # and the PE can run up to 6 chunks ahead of ACT
                    tq0 = HW * half
                    lo = max(0, 128 * tk - tq0)
                    chunks = ([(lo, 512), (512, HW)] if lo < 512
                              else [(lo, HW)])
                    for (a, bnd) in chunks:
                        ps = pB.tile([128, 512], f32, tag="s", bufs=5,
                                     name="ps_s")
                        nc.tensor.matmul(
                            ps[:, 0:bnd - a],
                            kt_sb[:, 128 * tk:128 * (tk + 1)],
                            qt_sb[h][:, tq0 + a:tq0 + bnd],
                            start=True, stop=True)
                        nc.scalar.activation(ptile[:, a:bnd], ps[:, 0:bnd - a],
                                             Exp, scale=SCALE)

                pend_tp = []

                def y_group(h, half, il, tiles, tp="defer"):
                    """One [Y|Z] accumulation + normalize + transpose-out.
                    tp="dma": xbar-DMA transpose on the sync queue — only
                    for groups whose O-proj consumer is far enough away to
                    ride out the serialized ~1.2us/transpose queue.
                    tp="defer": PE-transpose (ident matmul) + DVE evict,
                    deferred via pend_tp so it never waits on the DVE
                    normalize of its own group."""
                    gi = 8 * half + il
                    ps_yz = pB.tile([128, 512], f32, tag="b1", bufs=3,
                                    name="ps_yz")
                    for tk in range(gi + 1):
                        nc.tensor.matmul(
                            ps_yz[:, 0:HD + 1],
                            tiles[tk][:, 128 * il:128 * (il + 1)],
                            vaug_sb[:, tk, 0:HD + 1],
                            start=(tk == 0), stop=(tk == gi))
                    rz = ap.tile([128, 1], f32, tag="rz", bufs=8)
                    nc.vector.reciprocal(rz, ps_yz[:, HD:HD + 1])
                    ysb = ap.tile([128, HD], bf, tag="ysb", bufs=8)
                    nc.vector.tensor_scalar_mul(ysb, ps_yz[:, 0:HD], rz)
                    if tp == "dma":
                        nc.sync.dma_start_transpose(
                            yt_sb[h][:, 128 * gi:128 * (gi + 1)], ysb)
                    else:
                        pend_tp.append((h, gi, ysb))

                def flush_tp():
                    if not pend_tp:
                        return
                    h, gi, ysb = pend_tp.pop(0)
                    tp = pB.tile([128, 512], bf, tag="b1", bufs=3,
                                 name="tp")
                    nc.tensor.transpose(tp[:, 0:128], ysb, ident)
                    nc.vector.tensor_copy(
                        yt_sb[h][:, 128 * gi:128 * (gi + 1)], tp[:, 0:128])

                def och(ti, n, evict="dve", sub=None, ob_row=None):
                    """One O-proj psum chunk (quarter row tile) — the
                    granular PE filler unit (~0.85us).  Evictions stay off
                    ACT in exp-bound blocks; the pure-PE tail alternates
                    DVE/ACT and borrows the (idle) S^T psum ring for a
                    3-deep rotation.  sub=(a,b) narrows the chunk to cols
                    [512n+a, 512n+b); ob_row collects evictions into a
                    caller-owned row buffer (caller issues the DMA)."""
                    lo, hi = sub if sub is not None else (0, 512)
                    w = hi - lo
                    tsl = slice(128 * ti, 128 * (ti + 1))
                    nsl = slice(512 * n + lo, 512 * n + hi)
                    if evict == "mix":
                        ps_w = pB.tile([128, 512], f32, tag="s", bufs=5,
                                       name="ps_ot")
                        ps_o = ps_w[:, 0:w]
                    else:
                        ps_of = pB.tile([128, 512], f32, tag="b1",
                                        bufs=3, name="ps_o")
                        ps_o = ps_of[:, 0:w]
                    for h in range(NHL):
                        nc.tensor.matmul(
                            ps_o, yt_sb[h][:, tsl], wo_sb[:, h, nsl],
                            start=(h == 0), stop=(h == NHL - 1))
                    if ob_row is not None:
                        ob = ob_row[:, nsl]
                    else:
                        obf = ap.tile([128, 512], bf, tag="ob", bufs=12,
                                      name="obf")
                        ob = obf[:, 0:w]
                    # tail ("mix") evictions lean 3:1 on ACT — it is idle
                    # there while DVE still runs the last head's normalize
                    # and transpose-copy chains
                    if sub is not None and lo > 0:
                        # the final sliver evicts on ACT so it starts the
                        # moment its matmul retires (DVE is busy with the
                        # preceding chunk) and its DMA trigger chains on
                        # the same (scalar) sequencer
                        nc.scalar.copy(ob, ps_o)
                    elif evict == "dve" or (evict == "mix" and n % 4 == 1):
                        nc.vector.tensor_copy(ob, ps_o)
                    elif evict == "mix":
                        nc.scalar.copy(ob, ps_o)
                    else:
                        nc.vector.tensor_copy(ob, ps_o)
                    if ob_row is not None:
                        return
                    # late DMAs stay OFF the gpsimd queue: its end-of-
                    # context drain is ~5us when it still holds work,
                    # and every engine's final barrier waits on it
                    if evict == "mix":
                        nc.sync.dma_start(o[tsl, nsl], ob)
                    else:
                        nc.gpsimd.dma_start(o[tsl, nsl], ob)

                def oproj(ti, last=0):
                    """last=1: whole-row single DMA (fewer ~0.6us DIRECT2D
                    triggers on the sync sequencer near the end).  last=2:
                    final row — row DMA covers cols 0:1536 early, then the
                    last chunk splits 384+128 so the post-last-matmul
                    evict+DMA shadow is minimal."""
                    tsl = slice(128 * ti, 128 * (ti + 1))
                    if not last:
                        for n in range(C // 512):
                            och(ti, n, evict="mix")
                        return
                    # half-row segments issued right after the covering
                    # evict: the LAST transfer is small, so the context-
                    # end barrier (which waits on queue drain) isn't stuck
                    # behind a 512KB row transfer issued at the very end
                    obr = ap.tile([128, C], bf, tag="obr", bufs=2,
                                  name="obr")
                    for n in range(3 if last == 2 else 4):
                        och(ti, n, evict="mix", ob_row=obr)
                        if n == 1:
                            nc.sync.dma_start(o[tsl, 0:1024], obr[:, 0:1024])
                    if last == 1:
                        nc.sync.dma_start(o[tsl, 1024:C], obr[:, 1024:C])
                        return
                    nc.sync.dma_start(o[tsl, 1024:1536], obr[:, 1024:1536])
                    och(ti, 3, evict="mix", sub=(0, 384), ob_row=obr)
                    nc.sync.dma_start(o[tsl, 1536:1920], obr[:, 1536:1920])
                    och(ti, 3, evict="mix", sub=(384, 512), ob_row=obr)
                    nc.scalar.dma_start(o[tsl, 1920:2048], obr[:, 1920:2048])

                # p tiles: phase-A tiles for heads 0,1 half 0; fresh ring
                # tiles for everything else
                pt = {}
                pt[(0, 0)] = p0_sb[0]
                pt[(1, 0)] = p0_sb[1]

                def make_ptiles(h, half):
                    tiles = [ap.tile([128, HW], bf, tag="p", bufs=34,
                                     name=f"p_{h}_{half}_{tk}")
                             for tk in range(8 + 8 * half)]
                    pt[(h, half)] = tiles
                    return tiles

                # -- attention blocks, each S^T+exp window packed with PE
                #    filler sized tk-by-tk to the exp widths: Y groups are
                #    woven LARGEST-FIRST against the wide early exps, and
                #    O-proj rows are spread as quarter-row chunks.  Heads
                #    0/1 half-0 have no S^T here (exp'd in phase A) — their
                #    Y groups are the filler for the first two windows. --

                # half-0 Y groups: large il woven first (against the wide
                # early exps).  il<=2 PE-transpose (their O rows 0-2 come
                # too soon for the serialized DMA-transpose queue); il>=3
                # ride the sync-queue xbar DMA.
                def h0_mode(il):
                    return "dma" if il >= 3 else "defer"

                # block (2,0): S^T(2,0) + y(0,0), y(1,0) filler (reversed).
                # A few warm matmuls from a b1-ring tile (whose bank freed
                # with the bridge) pad the first tks, where the S tiles
                # still wait on phase A's last psum evictions — without
                # them HAM drops to half rate for ~7us here.
                warm3 = pB.tile([128, 512], f32, tag="b1", bufs=3,
                                name="warm3")
                t20 = make_ptiles(2, 0)
                for tk in range(8):
                    s_tile_B(2, 0, tk, t20[tk])
                    if tk == 0:
                        # all warm3 accesses complete before any y_group
                        # allocates from the b1 ring (no future-WAR)
                        for _ in range(6):
                            nc.tensor.matmul(warm3[0:1, :], kt_sb[:, 0:1],
                                             kt_sb[:, 0:512],
                                             start=True, stop=True)
                    y_group(0, 0, 7 - tk, pt[(0, 0)], h0_mode(7 - tk))
                    y_group(1, 0, 7 - tk, pt[(1, 0)], h0_mode(7 - tk))
                    flush_tp()

                # block (3,0): S^T(3,0) + y(2,0) (reversed), y(3,0) (lag 2).
                # (Adding warm padding here too was tried: it halves the
                # residual HAM window but its own PE cost cancels the gain.)
                t30 = make_ptiles(3, 0)
                for tk in range(8):
                    s_tile_B(3, 0, tk, t30[tk])
                    y_group(2, 0, 7 - tk, pt[(2, 0)], h0_mode(7 - tk))
                    if tk >= 2:
                        y_group(3, 0, tk - 2, pt[(3, 0)], h0_mode(tk - 2))
                    flush_tp()

                # block (0,1): S^T(0,1) + O rows 0-2 as chunks + y(3,0) tail
                t01 = make_ptiles(0, 1)
                for tk in range(NT):
                    s_tile_B(0, 1, tk, t01[tk])
                    flush_tp()
                    if tk == 0:
                        y_group(3, 0, 6, pt[(3, 0)], "dma")
                    if tk == 1:
                        y_group(3, 0, 7, pt[(3, 0)], "dma")
                    # och filler from tk=2: the first ~4 tks otherwise
                    # leave ~0.5us/cycle of PE idle (S chunks + the two
                    # y(3,0) groups are under the exp-paced cycle).
                    # (tk=0 start was tried — flat within run noise)
                    if 2 <= tk <= 13:
                        och((tk - 2) // 4, (tk - 2) % 4)

                # blocks (h,1) for h=1..3: S^T + y(h-1,1) + O-row chunks.
                # The PE transposes (flush_tp) double as PE filler for
                # these ACT-bound windows — removing them tips the blocks
                # into stall/HAM-throttle spirals (measured +44us).
                for h in range(1, NHL):
                    tiles = make_ptiles(h, 1)
                    nch = [(3 + 2 * (h - 1) + n // 4, n % 4)
                           for n in range(8 if h < 3 else 4)]
                    for tk in range(NT):
                        s_tile_B(h, 1, tk, tiles[tk])
                        if tk % 2 == 1 and tk <= 11:
                            flush_tp()
                            y_group(h - 1, 1, tk // 2, pt[(h - 1, 1)])
                        if (tk == 15 or (tk % 2 == 0 and tk >= 2)) and nch:
                            ti, n = nch.pop(0)
                            och(ti, n)
                        if h < 3:
                            if tk == 13:
                                flush_tp()
                                y_group(h - 1, 1, 6, pt[(h - 1, 1)])
                            if tk == 15:
                                flush_tp()
                                y_group(h - 1, 1, 7, pt[(h - 1, 1)])
                        else:
                            if tk == 12:
                                flush_tp()
                                y_group(2, 1, 6, pt[(2, 1)])
                            if tk == 13:
                                flush_tp()
                                y_group(3, 1, 0, pt[(3, 1)])
                            if tk == 14:
                                flush_tp()
                                y_group(2, 1, 7, pt[(2, 1)])
                            if tk == 15:
                                flush_tp()
                                y_group(3, 1, 1, pt[(3, 1)])

                # -- tail: last head's remaining Y two O-rows ahead of the
                #    O-proj rows that consume them (pure PE, ACT idle) --
                for il in range(2, 8):
                    flush_tp()
                    y_group(3, 1, il, pt[(3, 1)])
                    oproj(il + 6)
                flush_tp()
                oproj(14, last=1)
                oproj(15, last=2)

    nc.finalize()
    return nc


def _tables():
    freqs = 1.0 / (10000.0 ** (np.arange(0, HD, 2, dtype=np.float32) / HD))
    t = np.arange(T, dtype=np.float32)
    emb = np.outer(t, freqs)                  # [T, 64]
    cos_t = np.cos(emb).T.astype(np.float32)  # [64, T]
    sin_t = np.sin(emb).T.astype(np.float32)
    cosT = np.ascontiguousarray(np.concatenate([cos_t, cos_t], 0)).astype(BF16)
    # halves swapped: row d holds the factor multiplying t0[(d+64)%128]
    # when writing t2[d ^ 64 half]; see rope_evict
    sinT = np.ascontiguousarray(np.concatenate([sin_t, -sin_t], 0)).astype(BF16)
    return cosT, sinT


def _get_nc():
    if "nc" not in _CACHE:
        _CACHE["nc"] = _build_nc()
    return _CACHE["nc"]


def kernel(x, Wq, Wk, Wv, Wo, _trace=False):
    from concourse.bass_utils import run_bass_kernel_spmd

    x = np.asarray(x, dtype=np.float32)
    cosT, sinT = _tables()

    def chunked(w):
        # [K, m] -> [128, K//128, m] (partition-major, contiguous)
        k, m = w.shape
        return np.ascontiguousarray(
            w.reshape(k // 128, 128, m).transpose(1, 0, 2)).astype(BF16)

    in_maps = []
    for core in range(8):
        b, g = divmod(core, 4)
        wq_g = Wq[:, 512 * g:512 * (g + 1)]
        in_maps.append({
            "xT": np.ascontiguousarray(x[b].T).astype(BF16),
            "wq": np.ascontiguousarray(np.stack(
                [chunked(wq_g[:, 128 * h:128 * (h + 1)]) for h in range(NHL)],
                axis=1)),
            "wk": chunked(Wk[:, 128 * g:128 * (g + 1)]),
            "wv": chunked(Wv[:, 128 * g:128 * (g + 1)]),
            "wo": chunked(Wo[512 * g:512 * (g + 1), :]),
            "cosT": cosT,
            "sinT": sinT,
        })

    nc = _get_nc()
    res = run_bass_kernel_spmd(nc, in_maps, list(range(8)), trace=_trace)
    parts = [res.results[c]["o_part"].astype(np.float32) for c in range(8)]
    out = np.empty((B, T, C), dtype=np.float32)
    for b in range(B):
        out[b] = parts[4 * b] + parts[4 * b + 1] + parts[4 * b + 2] + parts[4 * b + 3]
    if _trace:
        return out, res
    return out

